# revision 1
# baseline (speedup 1.0000x reference)
"""GNN message-passing layer on 8 Trainium2 NeuronCores.

Strategy: edges are bucketed by destination node (6250 nodes/core), so the
segment-sum is core-local and no collectives are needed.

Per core:
  phase 1: A|B tables  A = nf @ Wm1[0:64] + b_m1,  B = nf @ Wm1[64:128]
           (node-major in DRAM, computed from host-transposed node_feat)
  phase 2: per 128-node block (49 blocks x 22 tiles x 128 edges, padded):
           gather A[src], B[dst] (indirect DMA), EF @ Wm1[128:192] on PE
           (host-pretransposed edge_feat tiles as stationary), add + SiLU,
           then segment-sum via one-hot matmul accumulating into PSUM.
           The one-hot is built on DVE from within-block dst indices.
  phase 3: aggregated = Wm2^T @ H + b_m2*deg (deg from host bincount),
           update MLP feature-major (biases are per-partition), residual,
           LayerNorm via PE ones-matmul stats + K=1 broadcast matmuls.

Output is written feature-major [64, 6272] per core; host transposes and
concatenates.
"""
import sys
sys.path.insert(0, "/opt/trn_rl_repo")
import numpy as np

import concourse.bass as bass
import concourse.bacc as bacc
import concourse.mybir as mybir
import concourse.tile as tile
from concourse.bass_utils import run_bass_kernel_spmd

F32 = mybir.dt.float32
I32 = mybir.dt.int32

N_NODES = 50000
N_EDGES = 1000000
D = 64
NC = 8
NPC = 6250            # nodes per core
BLOCKS = 49           # 49 * 128 = 6272 local node slots
TPB = 22              # tiles (of 128 edges) per block
NLOC = BLOCKS * 128   # 6272
TILES = BLOCKS * TPB  # 1078
EPC = TILES * 128     # 137984 padded edges per core
NPAD = 50176          # 392 * 128 node slots for the A/B table
LN_EPS = 1e-5

_CACHED = {}


def _build_bass():
    nc = bacc.Bacc("TRN2", target_bir_lowering=False, debug=False, num_devices=NC)

    # ---- I/O ----
    nf_t = nc.dram_tensor("nf_t", [65, NPAD], F32, kind="ExternalInput")
    nf_loc = nc.dram_tensor("nf_loc", [D, NLOC], F32, kind="ExternalInput")
    ef_t = nc.dram_tensor("ef_t", [TILES, D, 128], F32, kind="ExternalInput")
    isrc = nc.dram_tensor("isrc", [128, TILES], I32, kind="ExternalInput")
    idst = nc.dram_tensor("idst", [128, TILES], I32, kind="ExternalInput")
    dstr = nc.dram_tensor("dstr", [128, TILES], F32, kind="ExternalInput")
    deg = nc.dram_tensor("deg", [1, NLOC], F32, kind="ExternalInput")
    iota = nc.dram_tensor("iota", [128, 128], F32, kind="ExternalInput")
    w_ab = nc.dram_tensor("w_ab", [65, 128], F32, kind="ExternalInput")
    w_e = nc.dram_tensor("w_e", [D, D], F32, kind="ExternalInput")
    w_m2e = nc.dram_tensor("w_m2e", [65, D], F32, kind="ExternalInput")
    w_u1 = nc.dram_tensor("w_u1", [D, D], F32, kind="ExternalInput")
    b_u1 = nc.dram_tensor("b_u1", [D, 1], F32, kind="ExternalInput")
    w_u2e = nc.dram_tensor("w_u2e", [65, D], F32, kind="ExternalInput")
    gam = nc.dram_tensor("gam", [D, 1], F32, kind="ExternalInput")
    bet = nc.dram_tensor("bet", [D, 1], F32, kind="ExternalInput")
    out_fm = nc.dram_tensor("out_fm", [D, NLOC], F32, kind="ExternalOutput")

    with tile.TileContext(nc) as tc:
        with (
            tc.tile_pool(name="dram", bufs=1, space="DRAM") as dpool,
            tc.tile_pool(name="persist", bufs=1) as pp,
        ):
            a_t = dpool.tile([NPAD, D], F32, tag="a_t")
            b_t = dpool.tile([NPAD, D], F32, tag="b_t")

            # persistent SBUF state
            isrc_sb = pp.tile([128, TILES], I32, tag="isrc")
            idst_sb = pp.tile([128, TILES], I32, tag="idst")
            dstr_sb = pp.tile([128, TILES], F32, tag="dstr")
            iota_sb = pp.tile([128, 128], F32, tag="iota")
            wab_sb = pp.tile([65, 128], F32, tag="wab")
            we_sb = pp.tile([D, D], F32, tag="we")
            wm2_sb = pp.tile([65, D], F32, tag="wm2")
            wu1_sb = pp.tile([D, D], F32, tag="wu1")
            bu1_sb = pp.tile([D, 1], F32, tag="bu1")
            wu2_sb = pp.tile([65, D], F32, tag="wu2")
            gam_sb = pp.tile([D, 1], F32, tag="gam")
            bet_sb = pp.tile([D, 1], F32, tag="bet")
            hall = pp.tile([65, NLOC], F32, tag="hall")
            oinv = pp.tile([D, 1], F32, tag="oinv")   # 1/64 column
            eps_sb = pp.tile([1, 1], F32, tag="eps")
            ones1 = pp.tile([1, D], F32, tag="ones1")  # row of 1.0

            nc.sync.dma_start(isrc_sb[:], isrc[:])
            nc.sync.dma_start(idst_sb[:], idst[:])
            nc.sync.dma_start(dstr_sb[:], dstr[:])
            nc.sync.dma_start(iota_sb[:], iota[:])
            nc.sync.dma_start(wab_sb[:], w_ab[:])
            nc.sync.dma_start(we_sb[:], w_e[:])
            nc.sync.dma_start(wm2_sb[:], w_m2e[:])
            nc.sync.dma_start(wu1_sb[:], w_u1[:])
            nc.sync.dma_start(bu1_sb[:], b_u1[:])
            nc.sync.dma_start(wu2_sb[:], w_u2e[:])
            nc.sync.dma_start(gam_sb[:], gam[:])
            nc.sync.dma_start(bet_sb[:], bet[:])
            nc.sync.dma_start(hall[64:65, :], deg[:])
            nc.gpsimd.memset(oinv[:], 1.0 / 64.0)
            nc.gpsimd.memset(eps_sb[:], LN_EPS)
            nc.gpsimd.memset(ones1[:], 1.0)

            # ---------------- phase 1: A/B tables ----------------
            with (
                tc.tile_pool(name="p1", bufs=3) as p1,
                tc.tile_pool(name="p1ps", bufs=2, space="PSUM") as p1ps,
            ):
                for nb in range(NPAD // 128):
                    sl = slice(nb * 128, (nb + 1) * 128)
                    nfc = p1.tile([65, 128], F32, tag="nfc")
                    nc.sync.dma_start(nfc[:], nf_t[:, sl])
                    ps = p1ps.tile([128, 128], F32, tag="ab")
                    nc.tensor.matmul(ps[:], lhsT=nfc[:], rhs=wab_sb[:],
                                     start=True, stop=True)
                    ab = p1.tile([128, 128], F32, tag="ab_sb")
                    nc.vector.tensor_copy(ab[:], ps[:])
                    nc.sync.dma_start(a_t[:].rearrange("n d -> n d")[sl, :], ab[:, 0:D])
                    nc.sync.dma_start(b_t[:][sl, :], ab[:, D:2 * D])

            # ---------------- phase 2: edges ----------------
            with (
                tc.tile_pool(name="p2", bufs=3) as p2,
                tc.tile_pool(name="p2b", bufs=2) as p2b,
                tc.tile_pool(name="p2ps", bufs=2, space="PSUM") as p2ps,
                tc.tile_pool(name="p2ph", bufs=2, space="PSUM") as p2ph,
            ):
                for b in range(BLOCKS):
                    tsl = slice(b * TPB, (b + 1) * TPB)
                    ga = p2.tile([128, TPB * D], F32, tag="ga")
                    gb = p2.tile([128, TPB * D], F32, tag="gb")
                    for t in range(TPB):
                        gt = b * TPB + t
                        nc.gpsimd.indirect_dma_start(
                            out=ga[:, t * D:(t + 1) * D], out_offset=None,
                            in_=a_t[:],
                            in_offset=bass.IndirectOffsetOnAxis(
                                ap=isrc_sb[:, gt:gt + 1], axis=0),
                        )
                        nc.gpsimd.indirect_dma_start(
                            out=gb[:, t * D:(t + 1) * D], out_offset=None,
                            in_=b_t[:],
                            in_offset=bass.IndirectOffsetOnAxis(
                                ap=idst_sb[:, gt:gt + 1], axis=0),
                        )
                    nc.vector.tensor_add(ga[:], ga[:], gb[:])

                    ef = p2.tile([D, TPB * 128], F32, tag="ef")
                    nc.sync.dma_start(
                        ef[:].rearrange("f (t x) -> f t x", t=TPB),
                        ef_t[tsl, :, :].rearrange("t f x -> f t x"),
                    )

                    oh = p2b.tile([128, TPB * 128], F32, tag="oh")
                    nc.vector.tensor_tensor(
                        out=oh[:].rearrange("p (t x) -> p t x", t=TPB),
                        in0=iota_sb[:, None, :].to_broadcast([128, TPB, 128]),
                        in1=dstr_sb[:, tsl, None].to_broadcast([128, TPB, 128]),
                        op=mybir.AluOpType.is_equal,
                    )

                    pre = p2b.tile([128, TPB * D], F32, tag="pre")
                    # EF matmuls in groups of 8 tiles -> one PSUM bank
                    for g in range((TPB + 7) // 8):
                        t0, t1 = g * 8, min((g + 1) * 8, TPB)
                        ps8 = p2ps.tile([128, 512], F32, tag="ps8")
                        for t in range(t0, t1):
                            nc.tensor.matmul(
                                ps8[:, (t - t0) * D:(t - t0 + 1) * D],
                                lhsT=ef[:, t * 128:(t + 1) * 128],
                                rhs=we_sb[:],
                                start=True, stop=True, skip_group_check=True,
                            )
                        nc.vector.tensor_add(
                            pre[:, t0 * D:t1 * D],
                            ps8[:, 0:(t1 - t0) * D],
                            ga[:, t0 * D:t1 * D],
                        )
                    h = p2b.tile([128, TPB * D], F32, tag="h")
                    nc.scalar.activation(h[:], pre[:],
                                         mybir.ActivationFunctionType.Silu)

                    psH = p2ph.tile([D, 128], F32, tag="psH")
                    for t in range(TPB):
                        nc.tensor.matmul(
                            psH[:],
                            lhsT=h[:, t * D:(t + 1) * D],
                            rhs=oh[:, t * 128:(t + 1) * 128],
                            start=(t == 0), stop=(t == TPB - 1),
                        )
                    nc.vector.tensor_copy(hall[0:D, b * 128:(b + 1) * 128], psH[:])

            # ---------------- phase 3: update MLP + LayerNorm ----------------
            with (
                tc.tile_pool(name="p3", bufs=2) as p3,
                tc.tile_pool(name="p3ps", bufs=1, space="PSUM") as p3ps,
            ):
                starts = list(range(0, NLOC, 512))
                for cs in starts:
                    w = min(512, NLOC - cs)
                    sl = slice(cs, cs + w)
                    ps_a = p3ps.tile([D, 512], F32, tag="ps_a")
                    nc.tensor.matmul(ps_a[:, :w], lhsT=wm2_sb[:], rhs=hall[:, sl],
                                     start=True, stop=True)
                    agg = p3.tile([D, 512], F32, tag="agg")
                    nc.vector.tensor_copy(agg[:, :w], ps_a[:, :w])

                    ps_u1 = p3ps.tile([D, 512], F32, tag="ps_u1")
                    nc.tensor.matmul(ps_u1[:, :w], lhsT=wu1_sb[:], rhs=agg[:, :w],
                                     start=True, stop=True)
                    s1 = p3.tile([65, 512], F32, tag="s1")
                    nc.gpsimd.memset(s1[64:65, :w], 1.0)
                    nc.scalar.activation(s1[0:D, :w], ps_u1[:, :w],
                                         mybir.ActivationFunctionType.Silu,
                                         bias=bu1_sb[:])
                    ps_u2 = p3ps.tile([D, 512], F32, tag="ps_u2")
                    nc.tensor.matmul(ps_u2[:, :w], lhsT=wu2_sb[:], rhs=s1[:, :w],
                                     start=True, stop=True)

                    nfl = p3.tile([D, 512], F32, tag="nfl")
                    nc.sync.dma_start(nfl[:, :w], nf_loc[:, sl])
                    xr = p3.tile([D, 512], F32, tag="xr")
                    nc.vector.tensor_add(xr[:, :w], ps_u2[:, :w], nfl[:, :w])

                    sq = p3.tile([D, 512], F32, tag="sq")
                    nc.scalar.activation(sq[:, :w], xr[:, :w],
                                         mybir.ActivationFunctionType.Square)
                    ps_s1 = p3ps.tile([1, 512], F32, tag="ps_s1")
                    nc.tensor.matmul(ps_s1[:, :w], lhsT=oinv[:], rhs=xr[:, :w],
                                     start=True, stop=True)
                    ps_s2 = p3ps.tile([1, 512], F32, tag="ps_s2")
                    nc.tensor.matmul(ps_s2[:, :w], lhsT=oinv[:], rhs=sq[:, :w],
                                     start=True, stop=True)
                    mean_sb = p3.tile([1, 512], F32, tag="mean_sb")
                    nc.vector.tensor_copy(mean_sb[:, :w], ps_s1[:, :w])
                    msq = p3.tile([1, 512], F32, tag="msq")
                    nc.vector.tensor_mul(msq[:, :w], mean_sb[:, :w], mean_sb[:, :w])
                    var = p3.tile([1, 512], F32, tag="var")
                    nc.vector.tensor_tensor(out=var[:, :w], in0=ps_s2[:, :w],
                                            in1=msq[:, :w],
                                            op=mybir.AluOpType.subtract)
                    std = p3.tile([1, 512], F32, tag="std")
                    nc.scalar.activation(std[:, :w], var[:, :w],
                                         mybir.ActivationFunctionType.Sqrt,
                                         bias=eps_sb[:])
                    rstd = p3.tile([1, 512], F32, tag="rstd")
                    nc.vector.reciprocal(rstd[:, :w], std[:, :w])

                    ps_mb = p3ps.tile([D, 512], F32, tag="ps_mb")
                    nc.tensor.matmul(ps_mb[:, :w], lhsT=ones1[:], rhs=mean_sb[:, :w],
                                     start=True, stop=True)
                    ps_rb = p3ps.tile([D, 512], F32, tag="ps_rb")
                    nc.tensor.matmul(ps_rb[:, :w], lhsT=ones1[:], rhs=rstd[:, :w],
                                     start=True, stop=True)

                    t1_ = p3.tile([D, 512], F32, tag="t1")
                    nc.vector.tensor_tensor(out=t1_[:, :w], in0=xr[:, :w],
                                            in1=ps_mb[:, :w],
                                            op=mybir.AluOpType.subtract)
                    t2_ = p3.tile([D, 512], F32, tag="t2")
                    nc.vector.tensor_mul(t2_[:, :w], t1_[:, :w], ps_rb[:, :w])
                    oc = p3.tile([D, 512], F32, tag="oc")
                    nc.scalar.activation(oc[:, :w], t2_[:, :w],
                                         mybir.ActivationFunctionType.Identity,
                                         bias=bet_sb[:], scale=gam_sb[:])
                    nc.sync.dma_start(out_fm[:, sl], oc[:, :w])

    nc.compile()
    return nc


def _prep(node_feat, edge_src, edge_dst, edge_feat,
          W_m1, b_m1, W_m2, b_m2, W_u1, b_u1, W_u2, b_u2,
          ln_gamma, ln_beta):
    """Host-side sharding: bucket+sort edges by dst, pad to fixed tiles."""
    order = np.argsort(edge_dst, kind="stable")
    sdst = edge_dst[order]

    nf_t = np.zeros((65, NPAD), np.float32)
    nf_t[0:D, 0:N_NODES] = node_feat.T
    nf_t[64, :] = 1.0

    w_ab = np.zeros((65, 128), np.float32)
    w_ab[0:D, 0:D] = W_m1[0:D]
    w_ab[0:D, D:2 * D] = W_m1[D:2 * D]
    w_ab[64, 0:D] = b_m1
    w_e = np.ascontiguousarray(W_m1[2 * D:3 * D])
    w_m2e = np.zeros((65, D), np.float32)
    w_m2e[0:D] = W_m2
    w_m2e[64] = b_m2
    w_u2e = np.zeros((65, D), np.float32)
    w_u2e[0:D] = W_u2
    w_u2e[64] = b_u2
    iota = np.tile(np.arange(128, dtype=np.float32), (128, 1))

    common = {
        "nf_t": nf_t, "iota": iota, "w_ab": w_ab, "w_e": w_e,
        "w_m2e": w_m2e, "w_u1": np.ascontiguousarray(W_u1),
        "b_u1": b_u1.reshape(D, 1).astype(np.float32), "w_u2e": w_u2e,
        "gam": ln_gamma.reshape(D, 1).astype(np.float32),
        "bet": ln_beta.reshape(D, 1).astype(np.float32),
    }

    in_maps = []
    for c in range(NC):
        lo, hi = c * NPC, (c + 1) * NPC
        e0, e1 = np.searchsorted(sdst, lo), np.searchsorted(sdst, hi)
        eidx = order[e0:e1]
        ldst = sdst[e0:e1] - lo                     # local dst in [0, NPC)

        isrc = np.zeros((128, TILES), np.int32)
        idst = np.zeros((128, TILES), np.int32)
        dstr = np.full((128, TILES), -1.0, np.float32)
        ef_tiles = np.zeros((TILES, 128, D), np.float32)

        bstart = np.searchsorted(ldst, np.arange(BLOCKS + 1) * 128)
        for b in range(BLOCKS):
            n = bstart[b + 1] - bstart[b]
            if n > TPB * 128:
                raise ValueError(f"block overflow: core {c} block {b}: {n}")
            sel = eidx[bstart[b]:bstart[b + 1]]
            rel = (ldst[bstart[b]:bstart[b + 1]] - b * 128).astype(np.float32)
            # slot k within block -> tile b*TPB + k//128, partition k%128
            t_of = b * TPB + np.arange(n) // 128
            p_of = np.arange(n) % 128
            isrc[p_of, t_of] = edge_src[sel]
            idst[p_of, t_of] = edge_dst[sel]
            dstr[p_of, t_of] = rel
            ef_tiles[t_of, p_of] = edge_feat[sel]

        degc = np.zeros((1, NLOC), np.float32)
        cnt = np.bincount(ldst, minlength=NPC).astype(np.float32)
        degc[0, 0:NPC] = cnt

        nf_loc = np.zeros((D, NLOC), np.float32)
        nhi = min(N_NODES, lo + NLOC)
        nf_loc[:, 0:nhi - lo] = node_feat[lo:nhi].T

        in_maps.append({
            **common,
            "nf_loc": nf_loc,
            "ef_t": np.ascontiguousarray(ef_tiles.transpose(0, 2, 1)),
            "isrc": isrc, "idst": idst, "dstr": dstr, "deg": degc,
        })
    return in_maps


def kernel(**inputs):
    inputs = {k: np.asarray(v) for k, v in inputs.items()}
    in_maps = _prep(**inputs)
    if "nc" not in _CACHED:
        _CACHED["nc"] = _build_bass()
    res = run_bass_kernel_spmd(_CACHED["nc"], in_maps, list(range(NC)))
    out = np.empty((N_NODES, D), np.float32)
    for c in range(NC):
        out[c * NPC:(c + 1) * NPC] = res.results[c]["out_fm"].T[0:NPC]
    return out


if __name__ == "__main__":
    rng = np.random.default_rng(1)
    sys.path.insert(0, "/root/problem")
    import reference
    inputs = {k: np.asarray(v) for k, v in reference.setup_inputs().items()}
    exp = np.asarray(reference.reference(**inputs))
    got = kernel(**inputs)
    err = np.abs(got - exp).max() / (np.abs(exp).max() + 1e-30)
    print("Relative error:", err)



# revision 8
# speedup vs baseline: 1.5922x; 1.5922x over previous
"""GNN message-passing layer on 8 Trainium2 NeuronCores.

Strategy: edges are bucketed by destination node (6250 nodes/core), so the
segment-sum is core-local and no collectives are needed.

Per core:
  phase 1: A table  A = nf @ Wm1[0:64] + b_m1  ([50176, 64] f32 in DRAM,
           node-major, from bf16 node features), B table for the 6272 local
           nodes ([6272, 64] f32 in DRAM).
  phase 2: edges in 49 blocks of 128 dst slots; each block split into a
           lo/hi src-range half (12 tiles of 128 edges each) so gather
           indices fit int16. Per chunk of 3 blocks: batched dma_gather of
           A[src] (2 gathers) and B[dst] (1 gather), EF @ Wm1[128:192] on
           PE (bf16), adds on DVE, SiLU on ACT, then segment-sum via
           one-hot matmul accumulating into PSUM.
  phase 3: aggregated = Wm2^T @ H + b_m2*deg (deg from host bincount),
           update MLP feature-major, residual, LayerNorm via PE
           ones-matmul stats + K=1 broadcast matmuls.

Output is written feature-major [64, 6272] per core; host transposes and
concatenates.
"""
import os
import sys
sys.path.insert(0, "/opt/trn_rl_repo")
import numpy as np
import ml_dtypes

_ABL = set(os.environ.get("KABL", "").split(","))  # debug ablation flags

import concourse.bass as bass
import concourse.bacc as bacc
import concourse.mybir as mybir
import concourse.tile as tile
from concourse.bass_utils import run_bass_kernel_spmd

F32 = mybir.dt.float32
BF16 = mybir.dt.bfloat16
I16 = mybir.dt.int16

N_NODES = 50000
N_EDGES = 1000000
D = 64
NC = 8
NPC = 6250              # nodes per core
BLOCKS = 49             # 49 * 128 = 6272 local node slots
TG = 12                 # tiles (of 128 edges) per (block, src-range half)
TPB = 2 * TG            # tiles per block
NLOC = BLOCKS * 128     # 6272
TILES = BLOCKS * TPB    # 1176
SLOTS = TILES * 128     # 150528 padded edge slots per core
NPAD = 50176            # 392 * 128 node slots for the A table
SPLIT = NPAD // 2       # 25088: src-range split so gather idx fits int16
NBMAX = 3
CHUNKS = [3] * 16 + [1]  # blocks per phase-2 chunk
LN_EPS = 1e-5

_CACHED = {}


def _build_bass():
    nc = bacc.Bacc("TRN2", target_bir_lowering=False, debug=False, num_devices=NC)

    # ---- I/O ----
    nf_t = nc.dram_tensor("nf_t", [65, NPAD], BF16, kind="ExternalInput")
    nfl_t = nc.dram_tensor("nfl_t", [65, NLOC], BF16, kind="ExternalInput")
    nf_loc = nc.dram_tensor("nf_loc", [D, NLOC], F32, kind="ExternalInput")
    ef_t = nc.dram_tensor("ef_t", [D, SLOTS], BF16, kind="ExternalInput")
    idx_all = nc.dram_tensor("idx_all", [128, SLOTS // 8], I16, kind="ExternalInput")
    dstr = nc.dram_tensor("dstr", [128, TILES], BF16, kind="ExternalInput")
    deg = nc.dram_tensor("deg", [1, NLOC], F32, kind="ExternalInput")
    iota = nc.dram_tensor("iota", [128, 128], BF16, kind="ExternalInput")
    w_a = nc.dram_tensor("w_a", [65, D], BF16, kind="ExternalInput")
    w_b = nc.dram_tensor("w_b", [65, D], BF16, kind="ExternalInput")
    w_e = nc.dram_tensor("w_e", [D, D], BF16, kind="ExternalInput")
    w_m2e = nc.dram_tensor("w_m2e", [65, D], F32, kind="ExternalInput")
    w_u1 = nc.dram_tensor("w_u1", [D, D], F32, kind="ExternalInput")
    b_u1 = nc.dram_tensor("b_u1", [D, 1], F32, kind="ExternalInput")
    w_u2e = nc.dram_tensor("w_u2e", [65, D], F32, kind="ExternalInput")
    gam = nc.dram_tensor("gam", [D, 1], F32, kind="ExternalInput")
    bet = nc.dram_tensor("bet", [D, 1], F32, kind="ExternalInput")
    out_fm = nc.dram_tensor("out_fm", [D, NLOC], F32, kind="ExternalOutput")

    with tile.TileContext(nc) as tc:
        with (
            tc.tile_pool(name="dram", bufs=1, space="DRAM") as dpool,
            tc.tile_pool(name="persist", bufs=1) as pp,
        ):
            a_t = dpool.tile([NPAD, D], F32, tag="a_t")
            b_t = dpool.tile([NLOC, D], F32, tag="b_t")

            # persistent SBUF state
            dstr_sb = pp.tile([128, TILES], BF16, tag="dstr")
            iota_sb = pp.tile([128, 128], BF16, tag="iota")
            wa_sb = pp.tile([65, D], BF16, tag="wa")
            wb_sb = pp.tile([65, D], BF16, tag="wb")
            we_sb = pp.tile([D, D], BF16, tag="we")
            wm2_sb = pp.tile([65, D], F32, tag="wm2")
            wu1_sb = pp.tile([D, D], F32, tag="wu1")
            bu1_sb = pp.tile([D, 1], F32, tag="bu1")
            wu2_sb = pp.tile([65, D], F32, tag="wu2")
            gam_sb = pp.tile([D, 1], F32, tag="gam")
            bet_sb = pp.tile([D, 1], F32, tag="bet")
            hall = pp.tile([65, NLOC], F32, tag="hall")
            oinv = pp.tile([D, 1], F32, tag="oinv")   # 1/64 column
            eps_sb = pp.tile([1, 1], F32, tag="eps")
            ones1 = pp.tile([1, D], F32, tag="ones1")  # row of 1.0

            nc.sync.dma_start(dstr_sb[:], dstr[:])
            nc.sync.dma_start(iota_sb[:], iota[:])
            nc.sync.dma_start(wa_sb[:], w_a[:])
            nc.sync.dma_start(wb_sb[:], w_b[:])
            nc.sync.dma_start(we_sb[:], w_e[:])
            nc.sync.dma_start(wm2_sb[:], w_m2e[:])
            nc.sync.dma_start(wu1_sb[:], w_u1[:])
            nc.sync.dma_start(bu1_sb[:], b_u1[:])
            nc.sync.dma_start(wu2_sb[:], w_u2e[:])
            nc.sync.dma_start(gam_sb[:], gam[:])
            nc.sync.dma_start(bet_sb[:], bet[:])
            nc.sync.dma_start(hall[64:65, :], deg[:])
            nc.gpsimd.memset(oinv[:], 1.0 / 64.0)
            nc.gpsimd.memset(eps_sb[:], LN_EPS)
            nc.gpsimd.memset(ones1[:], 1.0)

            # ---------------- phase 1: A table (global) + B table (local) ---
            with (
                tc.tile_pool(name="p1", bufs=2) as p1,
                tc.tile_pool(name="p1o", bufs=2) as p1o,
                tc.tile_pool(name="p1ps", bufs=2, space="PSUM") as p1ps,
            ):
                # A: 49 groups of 8 128-node chunks (1024 nodes per group)
                SLAB = 8 * 128
                for g in range(NPAD // SLAB):
                    nfc = p1.tile([65, SLAB], BF16, tag="nfc")
                    nc.sync.dma_start(nfc[:], nf_t[:, g * SLAB:(g + 1) * SLAB])
                    ps = p1ps.tile([128, 512], F32, tag="psa")
                    for cix in range(8):
                        nc.tensor.matmul(
                            ps[:, cix * D:(cix + 1) * D],
                            lhsT=nfc[:, cix * 128:(cix + 1) * 128],
                            rhs=wa_sb[:], start=True, stop=True,
                            skip_group_check=True,
                        )
                    ab = p1o.tile([128, 512], F32, tag="absb")
                    nc.vector.tensor_copy(ab[:], ps[:])
                    nc.sync.dma_start(
                        a_t[:][g * SLAB:(g + 1) * SLAB, :]
                        .rearrange("(c p) f -> p c f", p=128),
                        ab[:].rearrange("p (c f) -> p c f", f=D),
                    )
                # B: local nodes, 7 groups of 7 chunks
                nflc = p1.tile([65, NLOC], BF16, tag="nflc")
                nc.sync.dma_start(nflc[:], nfl_t[:])
                for g in range(7):
                    n0 = g * 7 * 128
                    ps = p1ps.tile([128, 512], F32, tag="psb")
                    for cix in range(7):
                        nc.tensor.matmul(
                            ps[:, cix * D:(cix + 1) * D],
                            lhsT=nflc[:, n0 + cix * 128:n0 + (cix + 1) * 128],
                            rhs=wb_sb[:], start=True, stop=True,
                            skip_group_check=True,
                        )
                    bb = p1o.tile([128, 512], F32, tag="bbsb")
                    nc.vector.tensor_copy(bb[:, :7 * D], ps[:, :7 * D])
                    nc.sync.dma_start(
                        b_t[:][n0:n0 + 7 * 128, :]
                        .rearrange("(c p) f -> p c f", p=128),
                        bb[:, :7 * D].rearrange("p (c f) -> p c f", f=D),
                    )

            # ---------------- phase 2: edges ----------------
            NTM = NBMAX * TPB      # tiles in a full chunk (72)
            with (
                tc.tile_pool(name="p2", bufs=2) as p2,
                tc.tile_pool(name="p2h", bufs=2) as p2h,
                tc.tile_pool(name="p2o", bufs=2) as p2o,
                tc.tile_pool(name="p2ps", bufs=2, space="PSUM") as p2ps,
                tc.tile_pool(name="p2ph", bufs=2, space="PSUM") as p2ph,
            ):
                t0 = 0      # global tile base of chunk
                for ci, nb in enumerate(CHUNKS):
                    nt = nb * TPB          # tiles in chunk
                    nh = nb * TG           # tiles per src-range region
                    io = t0 * 16           # idx_all col offset

                    idxc = p2.tile([128, NTM * 16], I16, tag="idx")
                    nc.sync.dma_start(idxc[:, :nt * 16],
                                      idx_all[:, io:io + nt * 16])

                    efc = p2.tile([D, NTM * 128], BF16, tag="ef")
                    nc.sync.dma_start(efc[:, :nt * 128],
                                      ef_t[:, t0 * 128:(t0 + nt) * 128])

                    ga = p2.tile([128, NTM * D], F32, tag="ga")
                    gb = p2.tile([128, NTM * D], F32, tag="gb")
                    gav = ga[:].rearrange("p (c f) -> p c f", f=D)
                    gbv = gb[:].rearrange("p (c f) -> p c f", f=D)
                    if "noga" in _ABL:
                        nc.gpsimd.memset(ga[:], 0.0)
                    else:
                        nc.gpsimd.dma_gather(
                            gav[:, 0:nh, :], a_t[:][0:SPLIT, :],
                            idxc[:, 0:nh * 8], nh * 128, nh * 128, D,
                            single_packet=False)
                        nc.gpsimd.dma_gather(
                            gav[:, nh:nt, :], a_t[:][SPLIT:NPAD, :],
                            idxc[:, nh * 8:nt * 8], nh * 128, nh * 128, D,
                            single_packet=False)
                    if "nogb" in _ABL:
                        nc.gpsimd.memset(gb[:], 0.0)
                    elif True:
                        if "splitgb" in _ABL:
                            nc.gpsimd.dma_gather(
                                gbv[:, 0:nh, :], b_t[:],
                                idxc[:, nt * 8:nt * 12], nh * 128, nh * 128, D,
                                single_packet=False)
                            nc.gpsimd.dma_gather(
                                gbv[:, nh:nt, :], b_t[:],
                                idxc[:, nt * 12:nt * 16], nh * 128, nh * 128, D,
                                single_packet=False)
                        else:
                            nc.gpsimd.dma_gather(
                                gbv[:, 0:nt, :], b_t[:],
                                idxc[:, nt * 8:nt * 16], nt * 128, nt * 128, D,
                                single_packet=False)
                    # A[src] + B[dst], in place
                    nc.vector.tensor_add(ga[:, :nt * D], ga[:, :nt * D],
                                         gb[:, :nt * D])

                    # one-hot [edge, slot] per chunk
                    oh = p2o.tile([128, NTM * 128], BF16, tag="oh")
                    nc.vector.tensor_tensor(
                        out=oh[:, :nt * 128].rearrange("p (t x) -> p t x", t=nt),
                        in0=iota_sb[:, None, :].to_broadcast([128, nt, 128]),
                        in1=dstr_sb[:, t0:t0 + nt, None]
                        .to_broadcast([128, nt, 128]),
                        op=mybir.AluOpType.is_equal,
                    )

                    h = p2h.tile([128, NTM * D], BF16, tag="h")
                    for half in range(2 * nb):   # one (block, range-half)
                        hb = half * TG           # chunk-tile base of the half
                        ps = p2ps.tile([128, TG * D], F32, tag="pre")
                        for t in range(TG):
                            nc.tensor.matmul(
                                ps[:, t * D:(t + 1) * D],
                                lhsT=efc[:, (hb + t) * 128:(hb + t + 1) * 128],
                                rhs=we_sb[:], start=True, stop=True,
                                skip_group_check=True,
                            )
                        presb = p2h.tile([128, TG * D], F32, tag="presb")
                        nc.vector.tensor_add(
                            presb[:], ps[:], ga[:, hb * D:(hb + TG) * D])
                        nc.scalar.activation(
                            h[:, hb * D:(hb + TG) * D], presb[:],
                            mybir.ActivationFunctionType.Silu)

                    # segment-sum per block via one-hot matmul
                    for b in range(nb):
                        gblk = t0 // TPB + b
                        psH = p2ph.tile([D, 128], F32, tag="psH")
                        tl = [b * TG + t for t in range(TG)] + \
                             [nb * TG + b * TG + t for t in range(TG)]
                        for j, tt in enumerate(tl):
                            nc.tensor.matmul(
                                psH[:],
                                lhsT=h[:, tt * D:(tt + 1) * D],
                                rhs=oh[:, tt * 128:(tt + 1) * 128],
                                start=(j == 0), stop=(j == len(tl) - 1),
                            )
                        nc.vector.tensor_copy(
                            hall[0:D, gblk * 128:(gblk + 1) * 128], psH[:])
                    t0 += nt

            # ---------------- phase 3: update MLP + LayerNorm ----------------
            with (
                tc.tile_pool(name="p3", bufs=2) as p3,
                tc.tile_pool(name="p3ps", bufs=1, space="PSUM") as p3ps,
            ):
                for cs in range(0, NLOC, 512):
                    w = min(512, NLOC - cs)
                    sl = slice(cs, cs + w)
                    ps_a = p3ps.tile([D, 512], F32, tag="ps_a")
                    nc.tensor.matmul(ps_a[:, :w], lhsT=wm2_sb[:], rhs=hall[:, sl],
                                     start=True, stop=True)
                    agg = p3.tile([D, 512], F32, tag="agg")
                    nc.vector.tensor_copy(agg[:, :w], ps_a[:, :w])

                    ps_u1 = p3ps.tile([D, 512], F32, tag="ps_u1")
                    nc.tensor.matmul(ps_u1[:, :w], lhsT=wu1_sb[:], rhs=agg[:, :w],
                                     start=True, stop=True)
                    s1 = p3.tile([65, 512], F32, tag="s1")
                    nc.gpsimd.memset(s1[64:65, :w], 1.0)
                    nc.scalar.activation(s1[0:D, :w], ps_u1[:, :w],
                                         mybir.ActivationFunctionType.Silu,
                                         bias=bu1_sb[:])
                    ps_u2 = p3ps.tile([D, 512], F32, tag="ps_u2")
                    nc.tensor.matmul(ps_u2[:, :w], lhsT=wu2_sb[:], rhs=s1[:, :w],
                                     start=True, stop=True)

                    nfl = p3.tile([D, 512], F32, tag="nfl")
                    nc.sync.dma_start(nfl[:, :w], nf_loc[:, sl])
                    xr = p3.tile([D, 512], F32, tag="xr")
                    nc.vector.tensor_add(xr[:, :w], ps_u2[:, :w], nfl[:, :w])

                    sq = p3.tile([D, 512], F32, tag="sq")
                    nc.scalar.activation(sq[:, :w], xr[:, :w],
                                         mybir.ActivationFunctionType.Square)
                    ps_s1 = p3ps.tile([1, 512], F32, tag="ps_s1")
                    nc.tensor.matmul(ps_s1[:, :w], lhsT=oinv[:], rhs=xr[:, :w],
                                     start=True, stop=True)
                    ps_s2 = p3ps.tile([1, 512], F32, tag="ps_s2")
                    nc.tensor.matmul(ps_s2[:, :w], lhsT=oinv[:], rhs=sq[:, :w],
                                     start=True, stop=True)
                    mean_sb = p3.tile([1, 512], F32, tag="mean_sb")
                    nc.vector.tensor_copy(mean_sb[:, :w], ps_s1[:, :w])
                    msq = p3.tile([1, 512], F32, tag="msq")
                    nc.vector.tensor_mul(msq[:, :w], mean_sb[:, :w], mean_sb[:, :w])
                    var = p3.tile([1, 512], F32, tag="var")
                    nc.vector.tensor_tensor(out=var[:, :w], in0=ps_s2[:, :w],
                                            in1=msq[:, :w],
                                            op=mybir.AluOpType.subtract)
                    std = p3.tile([1, 512], F32, tag="std")
                    nc.scalar.activation(std[:, :w], var[:, :w],
                                         mybir.ActivationFunctionType.Sqrt,
                                         bias=eps_sb[:])
                    rstd = p3.tile([1, 512], F32, tag="rstd")
                    nc.vector.reciprocal(rstd[:, :w], std[:, :w])

                    ps_mb = p3ps.tile([D, 512], F32, tag="ps_mb")
                    nc.tensor.matmul(ps_mb[:, :w], lhsT=ones1[:], rhs=mean_sb[:, :w],
                                     start=True, stop=True)
                    ps_rb = p3ps.tile([D, 512], F32, tag="ps_rb")
                    nc.tensor.matmul(ps_rb[:, :w], lhsT=ones1[:], rhs=rstd[:, :w],
                                     start=True, stop=True)

                    t1_ = p3.tile([D, 512], F32, tag="t1")
                    nc.vector.tensor_tensor(out=t1_[:, :w], in0=xr[:, :w],
                                            in1=ps_mb[:, :w],
                                            op=mybir.AluOpType.subtract)
                    t2_ = p3.tile([D, 512], F32, tag="t2")
                    nc.vector.tensor_mul(t2_[:, :w], t1_[:, :w], ps_rb[:, :w])
                    oc = p3.tile([D, 512], F32, tag="oc")
                    nc.scalar.activation(oc[:, :w], t2_[:, :w],
                                         mybir.ActivationFunctionType.Identity,
                                         bias=bet_sb[:], scale=gam_sb[:])
                    nc.sync.dma_start(out_fm[:, sl], oc[:, :w])

    nc.compile()
    return nc


def _pack_idx(idx):
    """dma_gather index packing: idx k at [k%16, k//16], tiled x8."""
    n = len(idx)
    p = np.zeros((16, n // 16), np.int16)
    p[np.arange(n) % 16, np.arange(n) // 16] = idx
    return np.tile(p, (8, 1))


def _prep(node_feat, edge_src, edge_dst, edge_feat,
          W_m1, b_m1, W_m2, b_m2, W_u1, b_u1, W_u2, b_u2,
          ln_gamma, ln_beta):
    """Host-side sharding: bucket+sort edges by dst, split each dst block
    by src range (lo/hi), pad to fixed tiles."""
    bf = lambda x: np.asarray(x, np.float32).astype(ml_dtypes.bfloat16)
    order = np.argsort(edge_dst, kind="stable")
    sdst = edge_dst[order]

    nf_t = np.zeros((65, NPAD), np.float32)
    nf_t[0:D, 0:N_NODES] = node_feat.T
    nf_t[64, :] = 1.0

    w_a = np.zeros((65, D), np.float32)
    w_a[0:D] = W_m1[0:D]
    w_a[64] = b_m1
    w_b = np.zeros((65, D), np.float32)
    w_b[0:D] = W_m1[D:2 * D]
    w_e = np.ascontiguousarray(W_m1[2 * D:3 * D])
    w_m2e = np.zeros((65, D), np.float32)
    w_m2e[0:D] = W_m2
    w_m2e[64] = b_m2
    w_u2e = np.zeros((65, D), np.float32)
    w_u2e[0:D] = W_u2
    w_u2e[64] = b_u2
    iota = np.tile(np.arange(128, dtype=np.float32), (128, 1))

    # chunk-ordered global tile base per (block, half), and gather-stream
    # slot base per block (lo and hi streams are separate, same offsets)
    tile_base, gbase = {}, {}
    t0 = g0 = 0
    for ci, nb in enumerate(CHUNKS):
        b0 = sum(CHUNKS[:ci])
        for bi in range(nb):
            tile_base[(b0 + bi, 0)] = t0 + bi * TG
            tile_base[(b0 + bi, 1)] = t0 + nb * TG + bi * TG
            gbase[b0 + bi] = g0 + bi * TG * 128
        t0 += nb * TPB
        g0 += nb * TG * 128

    common = {
        "nf_t": bf(nf_t), "iota": bf(iota),
        "w_a": bf(w_a), "w_b": bf(w_b), "w_e": bf(w_e),
        "w_m2e": w_m2e, "w_u1": np.ascontiguousarray(W_u1),
        "b_u1": b_u1.reshape(D, 1).astype(np.float32), "w_u2e": w_u2e,
        "gam": ln_gamma.reshape(D, 1).astype(np.float32),
        "bet": ln_beta.reshape(D, 1).astype(np.float32),
    }

    in_maps = []
    for c in range(NC):
        lo, hi = c * NPC, (c + 1) * NPC
        e0, e1 = np.searchsorted(sdst, lo), np.searchsorted(sdst, hi)
        eidx = order[e0:e1]
        ldst = sdst[e0:e1] - lo                     # local dst in [0, NPC)
        esrc = edge_src[eidx]

        idx_ga = np.zeros(SLOTS // 2, np.int16)     # lo-half gather idx
        idx_gh = np.zeros(SLOTS // 2, np.int16)     # hi-half gather idx
        idx_b = np.zeros(SLOTS, np.int16)           # B gather idx (by slot)
        dstr = np.full((128, TILES), -1.0, np.float32)
        ef_sl = np.zeros((SLOTS, D), np.float32)    # slot-major edge feats

        bstart = np.searchsorted(ldst, np.arange(BLOCKS + 1) * 128)
        for b in range(BLOCKS):
            sel = eidx[bstart[b]:bstart[b + 1]]
            rel = (ldst[bstart[b]:bstart[b + 1]] - b * 128).astype(np.float32)
            s = esrc[bstart[b]:bstart[b + 1]]
            ld = ldst[bstart[b]:bstart[b + 1]]
            for half in range(2):
                m = (s < SPLIT) if half == 0 else (s >= SPLIT)
                n = int(m.sum())
                if n > TG * 128:
                    raise ValueError(
                        f"half overflow: core {c} block {b} half {half}: {n}")
                tb = tile_base[(b, half)]
                slots = tb * 128 + np.arange(n)
                dstr[slots % 128, slots // 128] = rel[m]
                ef_sl[slots] = edge_feat[sel[m]]
                gslots = gbase[b] + np.arange(n)
                if half == 0:
                    idx_ga[gslots] = s[m].astype(np.int16)
                else:
                    idx_gh[gslots] = (s[m] - SPLIT).astype(np.int16)
                idx_b[slots] = ld[m].astype(np.int16)

        # pack per-chunk: [lo idxs | hi idxs | B idxs]
        idx_cols = []
        t0 = g0 = 0
        for ci, nb in enumerate(CHUNKS):
            nsl = nb * TG * 128
            idx_cols.append(_pack_idx(idx_ga[g0:g0 + nsl]))
            idx_cols.append(_pack_idx(idx_gh[g0:g0 + nsl]))
            idx_cols.append(_pack_idx(idx_b[t0 * 128:(t0 + nb * TPB) * 128]))
            g0 += nsl
            t0 += nb * TPB
        idx_all = np.concatenate(idx_cols, axis=1)
        assert idx_all.shape == (128, SLOTS // 8)

        degc = np.zeros((1, NLOC), np.float32)
        cnt = np.bincount(ldst, minlength=NPC).astype(np.float32)
        degc[0, 0:NPC] = cnt

        nf_loc = np.zeros((D, NLOC), np.float32)
        nhi = min(N_NODES, lo + NLOC)
        nf_loc[:, 0:nhi - lo] = node_feat[lo:nhi].T
        nfl_t = np.zeros((65, NLOC), np.float32)
        nfl_t[0:D] = nf_loc

        in_maps.append({
            **common,
            "nfl_t": bf(nfl_t),
            "nf_loc": nf_loc,
            "ef_t": bf(np.ascontiguousarray(ef_sl.T)),
            "idx_all": idx_all,
            "dstr": bf(dstr), "deg": degc,
        })
    return in_maps


def kernel(**inputs):
    inputs = {k: np.asarray(v) for k, v in inputs.items()}
    in_maps = _prep(**inputs)
    if "nc" not in _CACHED:
        _CACHED["nc"] = _build_bass()
    res = run_bass_kernel_spmd(_CACHED["nc"], in_maps, list(range(NC)))
    out = np.empty((N_NODES, D), np.float32)
    for c in range(NC):
        out[c * NPC:(c + 1) * NPC] = res.results[c]["out_fm"].T[0:NPC]
    return out


if __name__ == "__main__":
    sys.path.insert(0, "/root/problem")
    import reference
    inputs = {k: np.asarray(v) for k, v in reference.setup_inputs().items()}
    exp = np.asarray(reference.reference(**inputs))
    got = kernel(**inputs)
    err = np.abs(got - exp).max() / (np.abs(exp).max() + 1e-30)
    print("Relative error:", err)
